# revision 15
# baseline (speedup 1.0000x reference)
"""Trainium2 Bass kernel for nn_AttnResModule (pooling / memory-bound).

Computation (reference):
    inv_rms = rsqrt(mean(V*V, -1) + eps)        # [n,B,T,1]
    K = V * inv_rms
    logits = einsum('d,nbtd->nbt', query, K)
    w = softmax(logits, axis=0)                  # over stack axis n=4
    out = einsum('nbt,nbtd->btd', w, V)

Key algebra: logits = (query . V) * inv_rms, so per row we need only two
free-axis reductions (ssq, dot) plus a cross-n weighted combine.

Layout per 128-row tile (rows = flattened (b,t)):
    partition p = n*32 + r   (n = stack index, r = row-subgroup index)
    free axis  = (j, d), j in 0..3, d in 0..2047
    row(tile, j, r) = tile*128 + 4*r + j   -> each partition holds 4
    CONSECUTIVE rows of V (32KB contiguous per DMA descriptor)

  - stats reduce along the free axis (DVE fused mul+reduce via
    scalar_tensor_tensor; ACT square with accum_out)
  - rsqrt computed as exp(-0.5*ln(x)) so Ln/Exp share one ACT table set
  - softmax denominator = sum over n = sum over partition groups ->
    tiny TensorE matmul with constant mask A[p,m] = (p%32 == m%32)
  - combine out[row, d] = sum_n w[n,row] V[n,row,d] -> TensorE matmul with
    block-diagonal weights lhsT_j[p, m] = (m == 4*(p%32)+j) * e[p] * sinv[p]
    so out partition m = row tile*128 + m (contiguous store).
  - combine runs in float32r (full-rate PE; ~tf32 rounding of V and weights)

Sharding: data-parallel over rows; 8 cores x 2048 rows, no communication.
"""

import sys
from contextlib import ExitStack

import numpy as np

_TRN_REPO = "/opt/trn_rl_repo"
if _TRN_REPO not in sys.path:
    sys.path.insert(0, _TRN_REPO)

import concourse.bacc as bacc
import concourse.tile as tile
from concourse import mybir
from concourse.bass_utils import run_bass_kernel_spmd

N_STACK = 4
B = 4
T = 4096
D = 2048
N_CORES = 8
ROWS = B * T
ROWS_PER_CORE = ROWS // N_CORES
EPS = float(np.finfo(np.float32).eps)
F32 = mybir.dt.float32
F32R = mybir.dt.float32r


def build_nc(
    rows_per_core=ROWS_PER_CORE,
    d=D,
    mode="bf16",  # "bf16" | "f32r" | "f32"
    v_bufs=8,
    n_gpsimd_sq=0,
    lhs_on_gpsimd=False,
    grp=4,
):
    n = N_STACK
    assert rows_per_core % 128 == 0
    assert d % 1024 == 0
    ntiles = rows_per_core // 128
    nc = bacc.Bacc(
        "TRN2",
        target_bir_lowering=False,
        debug=False,
        enable_asserts=False,
    )
    BF16 = mybir.dt.bfloat16
    vdt = {"bf16": BF16, "f32r": F32R, "f32": F32}[mode]
    qdt = BF16 if mode == "bf16" else F32
    V = nc.dram_tensor("v", [n, rows_per_core, d], vdt, kind="ExternalInput")
    QREP = nc.dram_tensor("qrep", [128, d], qdt, kind="ExternalInput")
    DMASK = nc.dram_tensor("dmask", [n, 128, 128], qdt, kind="ExternalInput")
    AMASK = nc.dram_tensor("amask", [128, 128], F32, kind="ExternalInput")
    OUT = nc.dram_tensor("out", [rows_per_core, d], F32, kind="ExternalOutput")

    mult = mybir.AluOpType.mult
    AF = mybir.ActivationFunctionType

    with ExitStack() as ctx:
        tc = ctx.enter_context(tile.TileContext(nc))
        singles = ctx.enter_context(tc.tile_pool(name="singles", bufs=1))
        vpool = ctx.enter_context(tc.tile_pool(name="vpool", bufs=v_bufs))
        qvpool = ctx.enter_context(tc.tile_pool(name="qvpool", bufs=2))
        sqpool = ctx.enter_context(tc.tile_pool(name="sqpool", bufs=2))
        outpool = ctx.enter_context(tc.tile_pool(name="outpool", bufs=2))
        lhspool = ctx.enter_context(tc.tile_pool(name="lhspool", bufs=2 * n))
        small = ctx.enter_context(tc.tile_pool(name="small", bufs=8))
        psum_o = ctx.enter_context(tc.tile_pool(name="psum_o", bufs=3, space="PSUM"))
        psum_s = ctx.enter_context(tc.tile_pool(name="psum_s", bufs=2, space="PSUM"))

        q_t = singles.tile([128, d], qdt)
        nc.sync.dma_start(out=q_t[:, :], in_=QREP.ap())
        dm_t = singles.tile([128, n, 128], qdt)
        nc.sync.dma_start(out=dm_t[:, :, :], in_=DMASK.ap().rearrange("j p m -> p j m"))
        am_t = singles.tile([128, 128], F32)
        nc.sync.dma_start(out=am_t[:, :], in_=AMASK.ap())
        eps_t = singles.tile([128, 1], F32)
        nc.vector.memset(eps_t[:, :], EPS)
        zero_t = singles.tile([128, 1], F32)
        nc.vector.memset(zero_t[:, :], 0.0)

        GRP = grp if ntiles % grp == 0 else 1
        for it in range(ntiles):
            ig = it % GRP  # index within softmax group
            R = it * 128
            v_t = vpool.tile([128, n, d], vdt, tag="v", name=f"v{it}")
            for sn in range(n):
                # row(j, r) = R + 4r + j: (j, d) contiguous per partition
                # -> one contiguous descriptor per partition. Alternate HWDGE
                # (SP ring) and SWDGE (gpsimd ring) so each SDMA engine
                # interleaves two input streams -> hides per-partition
                # SBUF-write serialization
                dma_eng = nc.sync if sn % 2 == 0 else nc.gpsimd
                dma_eng.dma_start(
                    out=v_t[sn * 32 : (sn + 1) * 32, :, :],
                    in_=V.ap()[sn, R : R + 128, :].rearrange(
                        "(r j) d -> r j d", j=n
                    ),
                )

            # ---- stats: dot (DVE fused) and ssq (ACT square), accumulated
            # into [128, GRP*n] group tiles so the softmax smalls batch ----
            if ig == 0:
                dot_big = small.tile([128, GRP * n], F32, tag="dot", name=f"dot{it}")
                ssq_big = small.tile([128, GRP * n], F32, tag="ssq", name=f"ssq{it}")
                group_v = []
                group_R = []
            group_v.append(v_t)
            group_R.append(R)
            for j in range(n):
                vj = v_t[:, j, :]
                if mode == "f32r":
                    vj = vj.bitcast(F32)
                qv = qvpool.tile([128, d], qdt, tag="qv", name=f"qv{j}")
                nc.vector.scalar_tensor_tensor(
                    out=qv[:, :],
                    in0=vj,
                    scalar=1.0,
                    in1=q_t[:, :],
                    op0=mult,
                    op1=mult,
                    accum_out=dot_big[:, ig * n + j : ig * n + j + 1],
                )
            for j in range(n):
                vj = v_t[:, j, :]
                if mode == "f32r":
                    vj = vj.bitcast(F32)
                sq = sqpool.tile([128, d], qdt, tag="sq", name=f"sq{j}")
                nc.scalar.activation(
                    out=sq[:, :],
                    in_=vj,
                    func=AF.Square,
                    bias=zero_t[:, :],
                    scale=1.0,
                    accum_out=ssq_big[:, ig * n + j : ig * n + j + 1],
                )

            if ig != GRP - 1:
                continue

            # ---- softmax weights for the whole group, batched [128, GRP*n] ----
            # invrms = exp(-0.5 * ln(ssq/D + eps))
            gn = GRP * n
            lnm = small.tile([128, gn], F32, tag="lnm")
            nc.scalar.activation(
                out=lnm[:, :],
                in_=ssq_big[:, :],
                func=AF.Ln,
                bias=eps_t[:, :],
                scale=1.0 / d,
            )
            inv_all = small.tile([128, gn], F32, tag="inv")
            nc.scalar.activation(
                out=inv_all[:, :],
                in_=lnm[:, :],
                func=AF.Exp,
                bias=zero_t[:, :],
                scale=-0.5,
            )
            logits = small.tile([128, gn], F32, tag="logits")
            nc.vector.tensor_mul(
                out=logits[:, :], in0=dot_big[:, :], in1=inv_all[:, :]
            )
            e_all = small.tile([128, gn], F32, tag="e")
            nc.scalar.activation(
                out=e_all[:, :],
                in_=logits[:, :],
                func=AF.Exp,
                bias=zero_t[:, :],
                scale=1.0,
            )
            # s[p, t*n+j] = sum_n' e[n'*32 + p%32, t*n+j]  (PE broadcast-sum)
            s_ps = psum_s.tile([128, gn], F32, tag="sps")
            nc.tensor.matmul(
                s_ps[:, :], am_t[:, :], e_all[:, :], start=True, stop=True
            )
            sinv_all = small.tile([128, gn], F32, tag="sinv")
            nc.vector.reciprocal(out=sinv_all[:, :], in_=s_ps[:, :])

            # ---- per tile in group: block-diag weights, PE combine, store ----
            ldt = mybir.dt.bfloat16 if mode == "bf16" else vdt
            for tg in range(GRP):
                vg = group_v[tg]
                Rg = group_R[tg]
                lhs_list = []
                for j in range(n):
                    lhsT_j = lhspool.tile(
                        [128, 128], ldt, tag="lhs", name=f"lhs{tg}_{j}"
                    )
                    eng = nc.gpsimd if lhs_on_gpsimd else nc.vector
                    eng.tensor_scalar(
                        out=lhsT_j[:, :],
                        in0=dm_t[:, j, :],
                        scalar1=e_all[:, tg * n + j : tg * n + j + 1],
                        scalar2=sinv_all[:, tg * n + j : tg * n + j + 1],
                        op0=mult,
                        op1=mult,
                    )
                    lhs_list.append(lhsT_j)

                nhalf = d // 1024
                ps_halves = []
                for half in range(nhalf):
                    ps_half = psum_o.tile(
                        [128, 1024], F32, tag="ps", name=f"ps{tg}_{half}"
                    )
                    ps_halves.append(ps_half)
                for j in range(n):
                    lhs_ap = lhs_list[j][:, :]
                    for half in range(nhalf):
                        for c in range(2):
                            off = half * 1024 + c * 512
                            nc.tensor.matmul(
                                ps_halves[half][:, c * 512 : (c + 1) * 512],
                                lhs_ap,
                                vg[:, j, off : off + 512],
                                start=(j == 0),
                                stop=(j == n - 1),
                            )
                out_sb = outpool.tile([128, d], F32, tag="osb", name=f"osb{tg}")
                for half in range(nhalf):
                    h0 = half * 1024
                    nc.scalar.copy(
                        out=out_sb[:, h0 : h0 + 512],
                        in_=ps_halves[half][:, 0:512],
                    )
                    nc.vector.tensor_copy(
                        out=out_sb[:, h0 + 512 : h0 + 1024],
                        in_=ps_halves[half][:, 512:1024],
                    )
                nc.scalar.dma_start(
                    out=OUT.ap()[Rg : Rg + 128, :], in_=out_sb[:, :]
                )

    nc.compile()
    return nc


def make_masks(n=N_STACK):
    p = np.arange(128)
    dmask = np.zeros((n, 128, 128), np.float32)
    for j in range(n):
        dmask[j, p, n * (p % 32) + j] = 1.0
    amask = np.equal.outer(p % 32, p % 32).astype(np.float32)
    return dmask, amask


def make_in_maps(V_flat, query, rows_per_core, n_cores, mode="bf16"):
    import ml_dtypes

    dmask, amask = make_masks()
    npdt = ml_dtypes.bfloat16 if mode == "bf16" else np.float32
    dmask = dmask.astype(npdt)
    qrep = np.ascontiguousarray(
        np.broadcast_to(query.astype(npdt), (128, V_flat.shape[2]))
    )
    in_maps = []
    for c in range(n_cores):
        vc = np.ascontiguousarray(
            V_flat[:, c * rows_per_core : (c + 1) * rows_per_core, :].astype(npdt)
        )
        in_maps.append({"v": vc, "qrep": qrep, "dmask": dmask, "amask": amask})
    return in_maps


_CACHE = {}


def _get_nc():
    if "nc" not in _CACHE:
        _CACHE["nc"] = build_nc()
    return _CACHE["nc"]


def kernel(V, query):
    V = np.asarray(V, dtype=np.float32)
    query = np.asarray(query, dtype=np.float32)
    assert V.shape == (N_STACK, B, T, D)
    nc = _get_nc()
    V_flat = V.reshape(N_STACK, ROWS, D)
    in_maps = make_in_maps(V_flat, query, ROWS_PER_CORE, N_CORES)
    res = run_bass_kernel_spmd(nc, in_maps, core_ids=list(range(N_CORES)))
    out = np.concatenate(
        [res.results[c]["out"] for c in range(N_CORES)], axis=0
    )
    return out.reshape(B, T, D)


if __name__ == "__main__":
    rng = np.random.default_rng(0)
    V = rng.standard_normal((N_STACK, B, T, D), dtype=np.float32)
    q = (rng.standard_normal(D) * 0.01).astype(np.float32)
    out = kernel(V, q)
    print("out", out.shape, out.dtype, float(np.abs(out).mean()))


# revision 16
# speedup vs baseline: 1.1114x; 1.1114x over previous
"""Trainium2 Bass kernel for nn_AttnResModule (pooling / memory-bound).

Computation (reference):
    inv_rms = rsqrt(mean(V*V, -1) + eps)        # [n,B,T,1]
    K = V * inv_rms
    logits = einsum('d,nbtd->nbt', query, K)
    w = softmax(logits, axis=0)                  # over stack axis n=4
    out = einsum('nbt,nbtd->btd', w, V)

Key algebra: logits = (query . V) * inv_rms, so per row we need only two
free-axis reductions (ssq, dot) plus a cross-n weighted combine.

Layout per 128-row tile (rows = flattened (b,t)):
    partition p = n*32 + r   (n = stack index, r = row-subgroup index)
    free axis  = (j, d), j in 0..3, d in 0..2047
    row(tile, j, r) = tile*128 + 4*r + j   -> each partition holds 4
    CONSECUTIVE rows of V (32KB contiguous per DMA descriptor)

  - stats reduce along the free axis (DVE fused mul+reduce via
    scalar_tensor_tensor; ACT square with accum_out)
  - rsqrt computed as exp(-0.5*ln(x)) so Ln/Exp share one ACT table set
  - softmax denominator = sum over n = sum over partition groups ->
    tiny TensorE matmul with constant mask A[p,m] = (p%32 == m%32)
  - combine out[row, d] = sum_n w[n,row] V[n,row,d] -> TensorE matmul with
    block-diagonal weights lhsT_j[p, m] = (m == 4*(p%32)+j) * e[p] * sinv[p]
    so out partition m = row tile*128 + m (contiguous store).
  - combine runs in float32r (full-rate PE; ~tf32 rounding of V and weights)

Sharding: data-parallel over rows; 8 cores x 2048 rows, no communication.
"""

import sys
from contextlib import ExitStack

import numpy as np

_TRN_REPO = "/opt/trn_rl_repo"
if _TRN_REPO not in sys.path:
    sys.path.insert(0, _TRN_REPO)

import concourse.bacc as bacc
import concourse.tile as tile
from concourse import mybir
from concourse.bass_utils import run_bass_kernel_spmd

N_STACK = 4
B = 4
T = 4096
D = 2048
N_CORES = 8
ROWS = B * T
ROWS_PER_CORE = ROWS // N_CORES
EPS = float(np.finfo(np.float32).eps)
F32 = mybir.dt.float32
F32R = mybir.dt.float32r


def build_nc(
    rows_per_core=ROWS_PER_CORE,
    d=D,
    mode="bf16",  # "bf16" | "f32r" | "f32"
    v_bufs=8,
    n_gpsimd_sq=0,
    lhs_on_gpsimd=False,
    grp=1,
):
    n = N_STACK
    assert rows_per_core % 128 == 0
    assert d % 1024 == 0
    ntiles = rows_per_core // 128
    nc = bacc.Bacc(
        "TRN2",
        target_bir_lowering=False,
        debug=False,
        enable_asserts=False,
    )
    BF16 = mybir.dt.bfloat16
    vdt = {"bf16": BF16, "f32r": F32R, "f32": F32}[mode]
    qdt = BF16 if mode == "bf16" else F32
    V = nc.dram_tensor("v", [n, rows_per_core, d], vdt, kind="ExternalInput")
    QREP = nc.dram_tensor("qrep", [128, d], qdt, kind="ExternalInput")
    DMASK = nc.dram_tensor("dmask", [n, 128, 128], qdt, kind="ExternalInput")
    AMASK = nc.dram_tensor("amask", [128, 128], F32, kind="ExternalInput")
    OUT = nc.dram_tensor("out", [rows_per_core, d], F32, kind="ExternalOutput")

    mult = mybir.AluOpType.mult
    AF = mybir.ActivationFunctionType

    with ExitStack() as ctx:
        tc = ctx.enter_context(tile.TileContext(nc))
        singles = ctx.enter_context(tc.tile_pool(name="singles", bufs=1))
        vpool = ctx.enter_context(tc.tile_pool(name="vpool", bufs=v_bufs))
        qvpool = ctx.enter_context(tc.tile_pool(name="qvpool", bufs=2))
        sqpool = ctx.enter_context(tc.tile_pool(name="sqpool", bufs=2))
        outpool = ctx.enter_context(tc.tile_pool(name="outpool", bufs=2))
        lhspool = ctx.enter_context(tc.tile_pool(name="lhspool", bufs=2 * n))
        small = ctx.enter_context(tc.tile_pool(name="small", bufs=8))
        psum_o = ctx.enter_context(tc.tile_pool(name="psum_o", bufs=3, space="PSUM"))
        psum_s = ctx.enter_context(tc.tile_pool(name="psum_s", bufs=2, space="PSUM"))

        q_t = singles.tile([128, d], qdt)
        nc.sync.dma_start(out=q_t[:, :], in_=QREP.ap())
        dm_t = singles.tile([128, n, 128], qdt)
        nc.sync.dma_start(out=dm_t[:, :, :], in_=DMASK.ap().rearrange("j p m -> p j m"))
        am_t = singles.tile([128, 128], F32)
        nc.sync.dma_start(out=am_t[:, :], in_=AMASK.ap())
        eps_t = singles.tile([128, 1], F32)
        nc.vector.memset(eps_t[:, :], EPS)
        zero_t = singles.tile([128, 1], F32)
        nc.vector.memset(zero_t[:, :], 0.0)

        GRP = grp if ntiles % grp == 0 else 1
        for it in range(ntiles):
            ig = it % GRP  # index within softmax group
            R = it * 128
            v_t = vpool.tile([128, n, d], vdt, tag="v", name=f"v{it}")
            for sn in range(n):
                # row(j, r) = R + 4r + j: (j, d) contiguous per partition
                # -> one contiguous descriptor per partition. Alternate HWDGE
                # (SP ring) and SWDGE (gpsimd ring) so each SDMA engine
                # interleaves two input streams -> hides per-partition
                # SBUF-write serialization
                dma_eng = nc.sync if sn % 2 == 0 else nc.gpsimd
                dma_eng.dma_start(
                    out=v_t[sn * 32 : (sn + 1) * 32, :, :],
                    in_=V.ap()[sn, R : R + 128, :].rearrange(
                        "(r j) d -> r j d", j=n
                    ),
                )

            # ---- stats: dot (DVE fused) and ssq (ACT square), accumulated
            # into [128, GRP*n] group tiles so the softmax smalls batch ----
            if ig == 0:
                dot_big = small.tile([128, GRP * n], F32, tag="dot", name=f"dot{it}")
                ssq_big = small.tile([128, GRP * n], F32, tag="ssq", name=f"ssq{it}")
                group_v = []
                group_R = []
            group_v.append(v_t)
            group_R.append(R)
            for j in range(n):
                vj = v_t[:, j, :]
                if mode == "f32r":
                    vj = vj.bitcast(F32)
                qv = qvpool.tile([128, d], qdt, tag="qv", name=f"qv{j}")
                nc.vector.scalar_tensor_tensor(
                    out=qv[:, :],
                    in0=vj,
                    scalar=1.0,
                    in1=q_t[:, :],
                    op0=mult,
                    op1=mult,
                    accum_out=dot_big[:, ig * n + j : ig * n + j + 1],
                )
            for j in range(n):
                vj = v_t[:, j, :]
                if mode == "f32r":
                    vj = vj.bitcast(F32)
                sq = sqpool.tile([128, d], qdt, tag="sq", name=f"sq{j}")
                nc.scalar.activation(
                    out=sq[:, :],
                    in_=vj,
                    func=AF.Square,
                    bias=zero_t[:, :],
                    scale=1.0,
                    accum_out=ssq_big[:, ig * n + j : ig * n + j + 1],
                )

            if ig != GRP - 1:
                continue

            # ---- softmax weights for the whole group, batched [128, GRP*n] ----
            # invrms = exp(-0.5 * ln(ssq/D + eps))
            gn = GRP * n
            lnm = small.tile([128, gn], F32, tag="lnm")
            nc.scalar.activation(
                out=lnm[:, :],
                in_=ssq_big[:, :],
                func=AF.Ln,
                bias=eps_t[:, :],
                scale=1.0 / d,
            )
            inv_all = small.tile([128, gn], F32, tag="inv")
            nc.scalar.activation(
                out=inv_all[:, :],
                in_=lnm[:, :],
                func=AF.Exp,
                bias=zero_t[:, :],
                scale=-0.5,
            )
            logits = small.tile([128, gn], F32, tag="logits")
            nc.vector.tensor_mul(
                out=logits[:, :], in0=dot_big[:, :], in1=inv_all[:, :]
            )
            e_all = small.tile([128, gn], F32, tag="e")
            nc.scalar.activation(
                out=e_all[:, :],
                in_=logits[:, :],
                func=AF.Exp,
                bias=zero_t[:, :],
                scale=1.0,
            )
            # s[p, t*n+j] = sum_n' e[n'*32 + p%32, t*n+j]  (PE broadcast-sum)
            s_ps = psum_s.tile([128, gn], F32, tag="sps")
            nc.tensor.matmul(
                s_ps[:, :], am_t[:, :], e_all[:, :], start=True, stop=True
            )
            sinv_all = small.tile([128, gn], F32, tag="sinv")
            nc.vector.reciprocal(out=sinv_all[:, :], in_=s_ps[:, :])

            # ---- per tile in group: block-diag weights, PE combine, store ----
            ldt = mybir.dt.bfloat16 if mode == "bf16" else vdt
            for tg in range(GRP):
                vg = group_v[tg]
                Rg = group_R[tg]
                lhs_list = []
                for j in range(n):
                    lhsT_j = lhspool.tile(
                        [128, 128], ldt, tag="lhs", name=f"lhs{tg}_{j}"
                    )
                    eng = nc.gpsimd if lhs_on_gpsimd else nc.vector
                    eng.tensor_scalar(
                        out=lhsT_j[:, :],
                        in0=dm_t[:, j, :],
                        scalar1=e_all[:, tg * n + j : tg * n + j + 1],
                        scalar2=sinv_all[:, tg * n + j : tg * n + j + 1],
                        op0=mult,
                        op1=mult,
                    )
                    lhs_list.append(lhsT_j)

                nhalf = d // 1024
                ps_halves = []
                for half in range(nhalf):
                    ps_half = psum_o.tile(
                        [128, 1024], F32, tag="ps", name=f"ps{tg}_{half}"
                    )
                    ps_halves.append(ps_half)
                for j in range(n):
                    lhs_ap = lhs_list[j][:, :]
                    for half in range(nhalf):
                        for c in range(2):
                            off = half * 1024 + c * 512
                            nc.tensor.matmul(
                                ps_halves[half][:, c * 512 : (c + 1) * 512],
                                lhs_ap,
                                vg[:, j, off : off + 512],
                                start=(j == 0),
                                stop=(j == n - 1),
                            )
                out_sb = outpool.tile([128, d], F32, tag="osb", name=f"osb{tg}")
                for half in range(nhalf):
                    h0 = half * 1024
                    nc.scalar.copy(
                        out=out_sb[:, h0 : h0 + 512],
                        in_=ps_halves[half][:, 0:512],
                    )
                    nc.vector.tensor_copy(
                        out=out_sb[:, h0 + 512 : h0 + 1024],
                        in_=ps_halves[half][:, 512:1024],
                    )
                nc.scalar.dma_start(
                    out=OUT.ap()[Rg : Rg + 128, :], in_=out_sb[:, :]
                )

    nc.compile()
    return nc


def make_masks(n=N_STACK):
    p = np.arange(128)
    dmask = np.zeros((n, 128, 128), np.float32)
    for j in range(n):
        dmask[j, p, n * (p % 32) + j] = 1.0
    amask = np.equal.outer(p % 32, p % 32).astype(np.float32)
    return dmask, amask


def make_in_maps(V_flat, query, rows_per_core, n_cores, mode="bf16"):
    import ml_dtypes

    dmask, amask = make_masks()
    npdt = ml_dtypes.bfloat16 if mode == "bf16" else np.float32
    dmask = dmask.astype(npdt)
    qrep = np.ascontiguousarray(
        np.broadcast_to(query.astype(npdt), (128, V_flat.shape[2]))
    )
    in_maps = []
    for c in range(n_cores):
        vc = np.ascontiguousarray(
            V_flat[:, c * rows_per_core : (c + 1) * rows_per_core, :].astype(npdt)
        )
        in_maps.append({"v": vc, "qrep": qrep, "dmask": dmask, "amask": amask})
    return in_maps


_CACHE = {}


def _get_nc():
    if "nc" not in _CACHE:
        _CACHE["nc"] = build_nc()
    return _CACHE["nc"]


def kernel(V, query):
    V = np.asarray(V, dtype=np.float32)
    query = np.asarray(query, dtype=np.float32)
    assert V.shape == (N_STACK, B, T, D)
    nc = _get_nc()
    V_flat = V.reshape(N_STACK, ROWS, D)
    in_maps = make_in_maps(V_flat, query, ROWS_PER_CORE, N_CORES)
    res = run_bass_kernel_spmd(nc, in_maps, core_ids=list(range(N_CORES)))
    out = np.concatenate(
        [res.results[c]["out"] for c in range(N_CORES)], axis=0
    )
    return out.reshape(B, T, D)


if __name__ == "__main__":
    rng = np.random.default_rng(0)
    V = rng.standard_normal((N_STACK, B, T, D), dtype=np.float32)
    q = (rng.standard_normal(D) * 0.01).astype(np.float32)
    out = kernel(V, q)
    print("out", out.shape, out.dtype, float(np.abs(out).mean()))
